# revision 2
# baseline (speedup 1.0000x reference)
"""Trainium2 Bass kernel for nn_BaseFullTensorProduct (e3nn-style full tensor product).

out[n] = FullTensorProduct(input1[n], input2[n]) over 19 CG paths:
  IRREPS1 = 64x0e + 64x1o + 32x2e (dim 416), IRREPS2 = 1x0e + 1x1o + 1x2e (dim 9),
  output dim 3744, N = 50000 rows, pure data-parallel over 8 NeuronCores.

Per-core pipeline (n-on-partitions layout, features on the free dim):
  stage 1 (TensorE):  A[n, (i,k)] = sum_j CGMAT[j, (i,k)] * x2[n, j]
      one matmul per 128-row block: stationary = x2^T tile [9 x 128],
      moving = constant CGMAT [9 x 315], out lands [128 x 315] in PSUM
      (n on partitions), evacuated to SBUF by ScalarE.
  stage 2 (VectorE):  out[n, u*d3+k] = sum_i x1[n, u*d1+i] * A[n, (i,k)]
      broadcast-AP tensor_tensor mult/add ops, one statically planned op list
      per CG path exploiting the structural zeros of the CG tensors; final ops
      write directly into the e3nn-sorted output column layout.
  Output tiles [128 x B*3744] stored with one contiguous DMA per chunk.
"""
import os
import numpy as np
from fractions import Fraction
from math import factorial

# ----------------------------------------------------------------------------
# Problem constants (hardcoded per spec: do not read spec.json / reference.py)
# ----------------------------------------------------------------------------
IRREPS1 = [(64, 0, 1), (64, 1, -1), (32, 2, 1)]
IRREPS2 = [(1, 0, 1), (1, 1, -1), (1, 2, 1)]
DIM1 = 416
DIM2 = 9
DIM_OUT = 3744
N_CORES = 8
BLOCK_P = 128          # rows per partition-block
B_BLOCKS = 4           # blocks fused per chunk (amortizes DVE per-op overhead)


# ----------------------------------------------------------------------------
# Clebsch-Gordan construction (e3nn real-basis convention, exact rationals)
# ----------------------------------------------------------------------------
def _fact(n):
    return factorial(round(n))


def _su2_cg(j1, j2, j3, m1, m2, m3):
    if m3 != m1 + m2:
        return 0.0
    vmin = int(max(-j1 + j2 + m3, -j1 + m1, 0))
    vmax = int(min(j2 + j3 + m1, j3 - j1 + j2, j3 + m3))
    C = ((2 * j3 + 1) * Fraction(
        _fact(j3 + j1 - j2) * _fact(j3 - j1 + j2) * _fact(j1 + j2 - j3)
        * _fact(j3 + m3) * _fact(j3 - m3),
        _fact(j1 + j2 + j3 + 1) * _fact(j1 - m1) * _fact(j1 + m1)
        * _fact(j2 - m2) * _fact(j2 + m2))) ** 0.5
    S = 0
    for v in range(vmin, vmax + 1):
        S += (-1) ** (v + j2 + m2) * Fraction(
            _fact(j2 + j3 + m1 - v) * _fact(j1 - m1 + v),
            _fact(v) * _fact(j3 - j1 + j2 - v) * _fact(j3 + m3 - v)
            * _fact(v + j1 - j2 - m3))
    return float(C * S)


def _su2_cg_tensor(j1, j2, j3):
    mat = np.zeros((2 * j1 + 1, 2 * j2 + 1, 2 * j3 + 1))
    for i, m1 in enumerate(range(-j1, j1 + 1)):
        for j, m2 in enumerate(range(-j2, j2 + 1)):
            if abs(m1 + m2) <= j3:
                mat[i, j, j3 + m1 + m2] = _su2_cg(j1, j2, j3, m1, m2, m1 + m2)
    return mat


def _q(l):
    q = np.zeros((2 * l + 1, 2 * l + 1), dtype=np.complex128)
    for m in range(-l, 0):
        q[l + m, l + abs(m)] = 1 / np.sqrt(2)
        q[l + m, l - abs(m)] = -1j / np.sqrt(2)
    q[l, l] = 1.0
    for m in range(1, l + 1):
        q[l + m, l + abs(m)] = (-1) ** m / np.sqrt(2)
        q[l + m, l - abs(m)] = 1j * (-1) ** m / np.sqrt(2)
    return (-1j) ** l * q


def _clebsch_gordan(l1, l2, l3):
    C = _su2_cg_tensor(l1, l2, l3).astype(np.complex128)
    C = np.einsum('ij,kl,mn,ikn->jlm', _q(l1), _q(l2), np.conj(_q(l3).T), C)
    C = np.real(C)
    return C / np.linalg.norm(C)


def _build_paths():
    paths = []
    off1 = 0
    for mul1, l1, p1 in IRREPS1:
        d1 = 2 * l1 + 1
        off2 = 0
        for mul2, l2, p2 in IRREPS2:
            d2 = 2 * l2 + 1
            for l3 in range(abs(l1 - l2), l1 + l2 + 1):
                cg = _clebsch_gordan(l1, l2, l3) * np.sqrt(2 * l3 + 1)
                paths.append((off1, mul1, d1, off2, mul2, d2, l3, p1 * p2, cg))
            off2 += mul2 * d2
        off1 += mul1 * d1
    order = [i for i, _ in sorted(
        enumerate(paths), key=lambda t: (t[1][6], -t[1][7] * (-1) ** t[1][6]))]
    return paths, order


PATHS, ORDER = _build_paths()

# ----------------------------------------------------------------------------
# Derived metadata: groups, A-matrix layout, CGMAT, output offsets, op plans
# ----------------------------------------------------------------------------
# Groups by l1 block of input1: (mul1, d1, x1 col base)
GROUPS = [(64, 1, 0), (64, 3, 64), (32, 5, 256)]
L1_OF_PATH = [{1: 0, 3: 1, 5: 2}[p[2]] for p in PATHS]

K_G = [0, 0, 0]
for p, pa in enumerate(PATHS):
    K_G[L1_OF_PATH[p]] += 2 * pa[6] + 1
assert K_G == [9, 27, 45]
A_BASE = [0, K_G[0] * 1, K_G[0] * 1 + K_G[1] * 3]
A_COLS = A_BASE[2] + K_G[2] * 5
assert A_COLS == 315

KO = []
_run = [0, 0, 0]
for p, pa in enumerate(PATHS):
    g = L1_OF_PATH[p]
    KO.append(_run[g])
    _run[g] += 2 * pa[6] + 1

OO = [0] * len(PATHS)
_off = 0
for p in ORDER:
    pa = PATHS[p]
    OO[p] = _off
    _off += pa[1] * (2 * pa[6] + 1)
assert _off == DIM_OUT


def _build_cgmat():
    m = np.zeros((DIM2, A_COLS), dtype=np.float32)
    for p, (s1, mul1, d1, s2, mul2, d2, l3, p3, cg) in enumerate(PATHS):
        g = L1_OF_PATH[p]
        d3 = 2 * l3 + 1
        for i in range(d1):
            for j2 in range(d2):
                for k3 in range(d3):
                    m[s2 + j2, A_BASE[g] + i * K_G[g] + KO[p] + k3] = cg[i, j2, k3]
    return m


CGMAT = _build_cgmat()


def _plan_path(p):
    """Static VectorE op plan for path p from the CG (i,k) sparsity.
    ops: ('diag',) | ('mw', i, a, ln) -> mult-write | ('mta', i, a, ln) -> mult-tmp+add
    """
    (s1, mul1, d1, s2, mul2, d2, l3, p3, cg) = PATHS[p]
    d3 = 2 * l3 + 1
    nz = (np.abs(cg) > 1e-12).any(axis=1)
    if d1 == d3 and np.array_equal(nz, np.eye(d1, dtype=bool)):
        return [('diag',)]
    ops = []
    written = np.zeros(d3, dtype=bool)
    order = sorted(range(d1), key=lambda i: -int(nz[i].sum()))
    for i in order:
        ks = np.flatnonzero(nz[i])
        if len(ks) == 0:
            continue
        runs = []
        a = prev = int(ks[0])
        for k in ks[1:]:
            k = int(k)
            if k == prev + 1:
                prev = k
            else:
                runs.append((a, prev - a + 1))
                a = prev = k
        runs.append((a, prev - a + 1))
        for (a, ln) in runs:
            j = a
            while j < a + ln:
                w = bool(written[j])
                e = j
                while e < a + ln and bool(written[e]) == w:
                    e += 1
                ops.append(('mta' if w else 'mw', i, j, e - j))
                j = e
        written[ks] = True
    assert written.all()
    return ops


PLANS = [_plan_path(p) for p in range(len(PATHS))]

# ----------------------------------------------------------------------------
# Bass kernel
# ----------------------------------------------------------------------------
_BUILD_CACHE = {}


def _patch_tile_drain():
    """The walrus in this env allows only ONE sem-wait per instruction
    ("Too many sync wait commands"); Tile freely emits multi-wait
    instructions. Split extra waits onto same-engine nops that execute
    just before the instruction (same semantics: engine streams are
    in-order, so waiting earlier on the same engine is equivalent)."""
    import bass_rust
    import concourse.mybir as mybir
    import concourse.tile as tile
    from concourse.vector_clock import ScopedClock

    if getattr(tile.TileContext, "_drain_split_patched", False):
        return

    _orig_add = tile.TileContext._add_instruction

    def _add_instruction(self, inst):
        si = inst.sync_info
        if si is not None and si.on_wait and len(si.on_wait) > 1:
            waits = list(si.on_wait)
            si.on_wait = waits[-1:]
            for w in waits[:-1]:
                nop = mybir.InstNoOp(
                    name=self.nc.get_next_instruction_name(), ins=[], outs=[])
                nop.engine = inst.engine
                nop.sync_info = bass_rust.SyncInfo(on_wait=[w], on_update=[])
                _orig_add(self, nop)
        _orig_add(self, inst)

    def _drain_and_barrier(self, tick_clock, wait_clock):
        holder = self.nc.sync.nop()
        wait_clock.add_sem_waits(
            holder.ins, ScopedClock({None: tick_clock.global_clock})
        )
        si = holder.ins.sync_info
        waits = list(si.on_wait or []) if si is not None else []
        if len(waits) > 1:
            si.on_wait = waits[:1]
            for w in waits[1:]:
                nop = self.nc.sync.nop()
                nop.ins.sync_info = bass_rust.SyncInfo(on_wait=[w], on_update=[])
        self.nc.sync.drain()
        self.nc.all_engine_barrier()
        assert self.sems is not None
        popped = self.nc._tile_sem_poison_stack.pop()
        assert popped is self._sem_poison
        self.nc.clear_and_free_semaphores(list(self.sems.allocated().values()))
        self.nc.all_engine_barrier()

    tile.TileContext._add_instruction = _add_instruction
    tile.TileContext._drain_and_barrier = _drain_and_barrier
    tile.TileContext._drain_split_patched = True


def _build_nc(rows):
    """Build the per-core Bass kernel for `rows` rows."""
    import concourse.bass as bass
    import concourse.mybir as mybir
    from concourse.tile import TileContext

    _patch_tile_drain()
    f32 = mybir.dt.float32
    mult = mybir.AluOpType.mult
    add = mybir.AluOpType.add

    nc = bass.Bass(num_devices=N_CORES)
    x1d = nc.declare_dram_parameter("input1", [rows, DIM1], f32, isOutput=False)
    x2d = nc.declare_dram_parameter("input2", [rows, DIM2], f32, isOutput=False)
    cgd = nc.declare_dram_parameter("cgmat", [DIM2, A_COLS], f32, isOutput=False)
    outd = nc.declare_dram_parameter("out", [rows, DIM_OUT], f32, isOutput=True)

    # chunk list: (row0, nb full blocks, p rows in last block)
    chunks = []
    r0 = 0
    while r0 < rows:
        nfull = min(B_BLOCKS, (rows - r0) // BLOCK_P)
        if nfull > 0:
            chunks.append((r0, nfull, BLOCK_P))
            r0 += nfull * BLOCK_P
        else:
            chunks.append((r0, 1, rows - r0))
            r0 = rows

    with TileContext(nc) as tc:
        with tc.tile_pool(name="const", bufs=1) as cpool, \
             tc.tile_pool(name="io", bufs=2) as iopool, \
             tc.tile_pool(name="tmp", bufs=4) as tmppool, \
             tc.tile_pool(name="psum", bufs=8, space="PSUM") as ppool:

            cgt = cpool.tile([DIM2, A_COLS], f32, tag="cg")
            nc.sync.dma_start(out=cgt[:], in_=cgd[:])

            for (row0, nb, pl) in chunks:
                # pl == BLOCK_P for all blocks except a lone partial chunk
                nrows = (nb - 1) * BLOCK_P + pl
                x1t = iopool.tile([BLOCK_P, nb * DIM1], f32, tag="x1")
                x2t = iopool.tile([DIM2, nb * BLOCK_P], f32, tag="x2t")
                at = iopool.tile([BLOCK_P, nb * A_COLS], f32, tag="A")
                outt = iopool.tile([BLOCK_P, nb * DIM_OUT], f32, tag="out")

                if pl == BLOCK_P:
                    src1 = x1d[row0:row0 + nrows, :].rearrange(
                        "(b n) f -> n b f", n=BLOCK_P)
                    src2 = x2d[row0:row0 + nrows, :].rearrange(
                        "(b n) j -> j b n", n=BLOCK_P)
                    dst_out = outd[row0:row0 + nrows, :].rearrange(
                        "(b n) f -> n b f", n=BLOCK_P)
                else:
                    src1 = x1d[row0:row0 + nrows, :].rearrange(
                        "(b n) f -> n b f", n=pl)
                    src2 = x2d[row0:row0 + nrows, :].rearrange(
                        "(b n) j -> j b n", n=pl)
                    dst_out = outd[row0:row0 + nrows, :].rearrange(
                        "(b n) f -> n b f", n=pl)

                x1v = x1t[:pl].rearrange("n (b f) -> n b f", b=nb)
                x2v = x2t[:].rearrange("j (b n) -> j b n", b=nb)[:, :, :pl]
                av = at[:pl].rearrange("n (b f) -> n b f", b=nb)
                outv = outt[:pl].rearrange("n (b f) -> n b f", b=nb)

                nc.sync.dma_start(out=x1v, in_=src1)
                nc.sync.dma_start(out=x2v, in_=src2)

                # stage 1: A = x2 @ CGMAT per block (TensorE), evac by ScalarE
                for b in range(nb):
                    pt = ppool.tile([BLOCK_P, A_COLS], f32, tag="apsum")
                    nc.tensor.matmul(
                        out=pt[:pl, :],
                        lhsT=x2v[:, b, :],
                        rhs=cgt[:],
                        start=True, stop=True)
                    nc.scalar.copy(out=av[:, b, :], in_=pt[:pl, :])

                # stage 2: per-path VectorE ops
                for p, (s1, mul1, d1, s2, mul2, d2, l3, p3, cg) in enumerate(PATHS):
                    g = L1_OF_PATH[p]
                    d3 = 2 * l3 + 1
                    Kg = K_G[g]
                    # out block view (n, b, u, k3)
                    ov = outv[:, :, OO[p]:OO[p] + mul1 * d3].rearrange(
                        "n b (u k) -> n b u k", k=d3)
                    # x1 block view (n, b, u, i)
                    xv = x1v[:, :, s1:s1 + mul1 * d1].rearrange(
                        "n b (u i) -> n b u i", i=d1)
                    # A rows for this group (n, b, i, k) ; path cols at KO[p]
                    gav = av[:, :, A_BASE[g]:A_BASE[g] + d1 * Kg].rearrange(
                        "n b (i k) -> n b i k", k=Kg)
                    for op in PLANS[p]:
                        if op[0] == 'diag':
                            # out[u,k] = x1[u,k] * A[k, KO+k]
                            o = ov[:, :, :, 0:d3]
                            xa = xv[:, :, :, 0:d1]
                            base = A_BASE[g] + KO[p]
                            aa = av[:, :, base: base + (d1 - 1) * (Kg + 1) + 1
                                    : Kg + 1]
                            aa = aa.unsqueeze(2).broadcast_to(
                                [pl, nb, mul1, d1])
                            nc.vector.tensor_tensor(o, xa, aa, mult)
                            continue
                        kind, i, a, ln = op
                        o = ov[:, :, :, a:a + ln]
                        xa = xv[:, :, :, i:i + 1].broadcast_to(
                            [pl, nb, mul1, ln])
                        aa = gav[:, :, i, KO[p] + a:KO[p] + a + ln]
                        aa = aa.unsqueeze(2).broadcast_to([pl, nb, mul1, ln])
                        if kind == 'mw':
                            nc.vector.tensor_tensor(o, xa, aa, mult)
                        else:
                            tt = tmppool.tile(
                                [BLOCK_P, nb * mul1 * ln], f32, tag="scratch")
                            tv = tt[:pl].rearrange(
                                "n (b u k) -> n b u k", u=mul1, k=ln)
                            nc.vector.tensor_tensor(tv, xa, aa, mult)
                            nc.vector.tensor_tensor(o, o, tv, add)

                nc.sync.dma_start(out=dst_out, in_=outv)

    return nc


def _get_nc(rows):
    if rows not in _BUILD_CACHE:
        _BUILD_CACHE[rows] = _build_nc(rows)
    return _BUILD_CACHE[rows]


def kernel(input1, input2):
    from concourse.bass_utils import run_bass_kernel_spmd

    input1 = np.ascontiguousarray(np.asarray(input1, dtype=np.float32))
    input2 = np.ascontiguousarray(np.asarray(input2, dtype=np.float32))
    n = input1.shape[0]
    rows = -(-n // N_CORES)  # ceil
    pad = rows * N_CORES - n
    if pad:
        input1 = np.concatenate(
            [input1, np.zeros((pad, DIM1), np.float32)], axis=0)
        input2 = np.concatenate(
            [input2, np.zeros((pad, DIM2), np.float32)], axis=0)

    nc = _get_nc(rows)
    in_maps = [
        {
            "input1": input1[c * rows:(c + 1) * rows],
            "input2": input2[c * rows:(c + 1) * rows],
            "cgmat": CGMAT,
        }
        for c in range(N_CORES)
    ]
    res = run_bass_kernel_spmd(nc, in_maps, core_ids=list(range(N_CORES)))
    out = np.concatenate([res.results[c]["out"] for c in range(N_CORES)], axis=0)
    return out[:n]


# revision 8
# speedup vs baseline: 1.9863x; 1.9863x over previous
"""Trainium2 Bass kernel for nn_BaseFullTensorProduct (e3nn-style full tensor product).

out[n] = FullTensorProduct(input1[n], input2[n]) over 19 CG paths:
  IRREPS1 = 64x0e + 64x1o + 32x2e (dim 416), IRREPS2 = 1x0e + 1x1o + 1x2e (dim 9),
  output dim 3744, N = 50000 rows, pure data-parallel over 8 NeuronCores.

Per-core pipeline (n-on-partitions layout, features on the free dim):
  stage 1 (TensorE):  A[n, (i,k)] = sum_j CGMAT[j, (i,k)] * x2[n, j]
      one matmul per 128-row block: stationary = x2^T tile [9 x 128],
      moving = constant CGMAT [9 x 315], out lands [128 x 315] in PSUM
      (n on partitions), evacuated to SBUF by ScalarE.
  stage 2 (VectorE):  out[n, u*d3+k] = sum_i x1[n, u*d1+i] * A[n, (i,k)]
      broadcast-AP tensor_tensor mult/add ops, one statically planned op list
      per CG path exploiting the structural zeros of the CG tensors; final ops
      write directly into the e3nn-sorted output column layout.
  Output tiles [128 x B*3744] stored with one contiguous DMA per chunk.
"""
import os
import numpy as np
from fractions import Fraction
from math import factorial

# ----------------------------------------------------------------------------
# Problem constants (hardcoded per spec: do not read spec.json / reference.py)
# ----------------------------------------------------------------------------
IRREPS1 = [(64, 0, 1), (64, 1, -1), (32, 2, 1)]
IRREPS2 = [(1, 0, 1), (1, 1, -1), (1, 2, 1)]
DIM1 = 416
DIM2 = 9
DIM_OUT = 3744
N_CORES = 8
BLOCK_P = 128          # rows per partition-block
B_BLOCKS = 5           # blocks fused per chunk (amortizes DVE per-op overhead)


# ----------------------------------------------------------------------------
# Clebsch-Gordan construction (e3nn real-basis convention, exact rationals)
# ----------------------------------------------------------------------------
def _fact(n):
    return factorial(round(n))


def _su2_cg(j1, j2, j3, m1, m2, m3):
    if m3 != m1 + m2:
        return 0.0
    vmin = int(max(-j1 + j2 + m3, -j1 + m1, 0))
    vmax = int(min(j2 + j3 + m1, j3 - j1 + j2, j3 + m3))
    C = ((2 * j3 + 1) * Fraction(
        _fact(j3 + j1 - j2) * _fact(j3 - j1 + j2) * _fact(j1 + j2 - j3)
        * _fact(j3 + m3) * _fact(j3 - m3),
        _fact(j1 + j2 + j3 + 1) * _fact(j1 - m1) * _fact(j1 + m1)
        * _fact(j2 - m2) * _fact(j2 + m2))) ** 0.5
    S = 0
    for v in range(vmin, vmax + 1):
        S += (-1) ** (v + j2 + m2) * Fraction(
            _fact(j2 + j3 + m1 - v) * _fact(j1 - m1 + v),
            _fact(v) * _fact(j3 - j1 + j2 - v) * _fact(j3 + m3 - v)
            * _fact(v + j1 - j2 - m3))
    return float(C * S)


def _su2_cg_tensor(j1, j2, j3):
    mat = np.zeros((2 * j1 + 1, 2 * j2 + 1, 2 * j3 + 1))
    for i, m1 in enumerate(range(-j1, j1 + 1)):
        for j, m2 in enumerate(range(-j2, j2 + 1)):
            if abs(m1 + m2) <= j3:
                mat[i, j, j3 + m1 + m2] = _su2_cg(j1, j2, j3, m1, m2, m1 + m2)
    return mat


def _q(l):
    q = np.zeros((2 * l + 1, 2 * l + 1), dtype=np.complex128)
    for m in range(-l, 0):
        q[l + m, l + abs(m)] = 1 / np.sqrt(2)
        q[l + m, l - abs(m)] = -1j / np.sqrt(2)
    q[l, l] = 1.0
    for m in range(1, l + 1):
        q[l + m, l + abs(m)] = (-1) ** m / np.sqrt(2)
        q[l + m, l - abs(m)] = 1j * (-1) ** m / np.sqrt(2)
    return (-1j) ** l * q


def _clebsch_gordan(l1, l2, l3):
    C = _su2_cg_tensor(l1, l2, l3).astype(np.complex128)
    C = np.einsum('ij,kl,mn,ikn->jlm', _q(l1), _q(l2), np.conj(_q(l3).T), C)
    C = np.real(C)
    return C / np.linalg.norm(C)


def _build_paths():
    paths = []
    off1 = 0
    for mul1, l1, p1 in IRREPS1:
        d1 = 2 * l1 + 1
        off2 = 0
        for mul2, l2, p2 in IRREPS2:
            d2 = 2 * l2 + 1
            for l3 in range(abs(l1 - l2), l1 + l2 + 1):
                cg = _clebsch_gordan(l1, l2, l3) * np.sqrt(2 * l3 + 1)
                paths.append((off1, mul1, d1, off2, mul2, d2, l3, p1 * p2, cg))
            off2 += mul2 * d2
        off1 += mul1 * d1
    order = [i for i, _ in sorted(
        enumerate(paths), key=lambda t: (t[1][6], -t[1][7] * (-1) ** t[1][6]))]
    return paths, order


PATHS, ORDER = _build_paths()

# ----------------------------------------------------------------------------
# Derived metadata: groups, A-matrix layout, CGMAT, output offsets, op plans
# ----------------------------------------------------------------------------
# Groups by l1 block of input1: (mul1, d1, x1 col base)
GROUPS = [(64, 1, 0), (64, 3, 64), (32, 5, 256)]
L1_OF_PATH = [{1: 0, 3: 1, 5: 2}[p[2]] for p in PATHS]

K_G = [0, 0, 0]
for p, pa in enumerate(PATHS):
    K_G[L1_OF_PATH[p]] += 2 * pa[6] + 1
assert K_G == [9, 27, 45]
A_BASE = [0, K_G[0] * 1, K_G[0] * 1 + K_G[1] * 3]
A_COLS = A_BASE[2] + K_G[2] * 5
assert A_COLS == 315

KO = []
_run = [0, 0, 0]
for p, pa in enumerate(PATHS):
    g = L1_OF_PATH[p]
    KO.append(_run[g])
    _run[g] += 2 * pa[6] + 1

OO = [0] * len(PATHS)
_off = 0
for p in ORDER:
    pa = PATHS[p]
    OO[p] = _off
    _off += pa[1] * (2 * pa[6] + 1)
assert _off == DIM_OUT


def _build_cgmat():
    m = np.zeros((DIM2, A_COLS), dtype=np.float32)
    for p, (s1, mul1, d1, s2, mul2, d2, l3, p3, cg) in enumerate(PATHS):
        g = L1_OF_PATH[p]
        d3 = 2 * l3 + 1
        for i in range(d1):
            for j2 in range(d2):
                for k3 in range(d3):
                    m[s2 + j2, A_BASE[g] + i * K_G[g] + KO[p] + k3] = cg[i, j2, k3]
    return m


CGMAT = _build_cgmat()


def _arith_runs(ks):
    """Cover a sorted k-set with maximal arithmetic runs, stride in {1, 2}."""
    ks = list(ks)
    runs = []
    while ks:
        a = ks[0]
        l1 = 1
        while a + l1 in ks:
            l1 += 1
        l2 = 1
        while a + 2 * l2 in ks:
            l2 += 1
        if l2 > l1:
            runs.append((a, l2, 2))
            for t in range(l2):
                ks.remove(a + 2 * t)
        else:
            runs.append((a, l1, 1))
            for t in range(l1):
                ks.remove(a + t)
    return runs


_DVE_INIT = 58  # per-op SBUF access bubble (cycles) used for plan costing


def _plan_path(p):
    """Static op plan for path p from the CG (i,k) sparsity.
    ops: [('act',)]                        -> l2=0: ScalarE out = x1_block * x2[:,0]
         [('dense',)]                      -> dense mult + tensor_reduce over i
         ('mw', i, a, ln, s)               -> TT mult-write, k = a, a+s, ...
         ('mta', i, a, ln, s)              -> TT mult-to-tmp + TT add
    """
    (s1, mul1, d1, s2, mul2, d2, l3, p3, cg) = PATHS[p]
    d3 = 2 * l3 + 1
    nz = (np.abs(cg) > 1e-12).any(axis=1)
    if d2 == 1:
        # cg for coupling with a scalar irrep is exactly the identity map
        assert np.allclose(cg[:, 0, :], np.eye(d1)), f"path {p} l2=0 cg != I"
        return [('act',)]
    ops = []
    written = np.zeros(d3, dtype=bool)
    order = sorted(range(d1), key=lambda i: -int(nz[i].sum()))
    for i in order:
        ks = [int(k) for k in np.flatnonzero(nz[i])]
        if not ks:
            continue
        for (a, ln, s) in _arith_runs(ks):
            j = 0
            while j < ln:
                w = bool(written[a + j * s])
                e = j
                while e < ln and bool(written[a + e * s]) == w:
                    e += 1
                ops.append(('mta' if w else 'mw', i, a + j * s, e - j, s))
                j = e
        for k in ks:
            written[k] = True
    assert written.all()
    # dense alternative: one mult over (u, k, i) + one tensor_reduce over i
    B = B_BLOCKS
    run_cost = sum(
        (2 if o[0] == 'mta' else 1) * (B * mul1 * o[3] + _DVE_INIT) for o in ops)
    dense_cost = 2 * (B * mul1 * d1 * d3) + 2 * _DVE_INIT
    if dense_cost < run_cost:
        return [('dense',)]
    return ops


PLANS = [_plan_path(p) for p in range(len(PATHS))]

# ----------------------------------------------------------------------------
# Bass kernel
# ----------------------------------------------------------------------------
_BUILD_CACHE = {}


def _patch_tile_drain():
    """The walrus in this env allows only ONE sem-wait per instruction
    ("Too many sync wait commands"); Tile freely emits multi-wait
    instructions. Split extra waits onto same-engine nops that execute
    just before the instruction (same semantics: engine streams are
    in-order, so waiting earlier on the same engine is equivalent)."""
    import bass_rust
    import concourse.mybir as mybir
    import concourse.tile as tile
    from concourse.vector_clock import ScopedClock

    if getattr(tile.TileContext, "_drain_split_patched", False):
        return

    _orig_add = tile.TileContext._add_instruction

    def _add_instruction(self, inst):
        si = inst.sync_info
        if si is not None and si.on_wait and len(si.on_wait) > 1:
            waits = list(si.on_wait)
            si.on_wait = waits[-1:]
            for w in waits[:-1]:
                nop = mybir.InstNoOp(
                    name=self.nc.get_next_instruction_name(), ins=[], outs=[])
                nop.engine = inst.engine
                nop.sync_info = bass_rust.SyncInfo(on_wait=[w], on_update=[])
                _orig_add(self, nop)
        _orig_add(self, inst)

    def _drain_and_barrier(self, tick_clock, wait_clock):
        holder = self.nc.sync.nop()
        wait_clock.add_sem_waits(
            holder.ins, ScopedClock({None: tick_clock.global_clock})
        )
        si = holder.ins.sync_info
        waits = list(si.on_wait or []) if si is not None else []
        if len(waits) > 1:
            si.on_wait = waits[:1]
            for w in waits[1:]:
                nop = self.nc.sync.nop()
                nop.ins.sync_info = bass_rust.SyncInfo(on_wait=[w], on_update=[])
        self.nc.sync.drain()
        self.nc.all_engine_barrier()
        assert self.sems is not None
        popped = self.nc._tile_sem_poison_stack.pop()
        assert popped is self._sem_poison
        self.nc.clear_and_free_semaphores(list(self.sems.allocated().values()))
        self.nc.all_engine_barrier()

    tile.TileContext._add_instruction = _add_instruction
    tile.TileContext._drain_and_barrier = _drain_and_barrier
    tile.TileContext._drain_split_patched = True


def _build_nc(rows):
    """Build the per-core Bass kernel for `rows` rows."""
    import concourse.bass as bass
    import concourse.mybir as mybir
    from concourse.tile import TileContext

    _patch_tile_drain()
    f32 = mybir.dt.float32
    mult = mybir.AluOpType.mult
    add = mybir.AluOpType.add

    nc = bass.Bass(num_devices=N_CORES)
    x1d = nc.declare_dram_parameter("input1", [rows, DIM1], f32, isOutput=False)
    x2d = nc.declare_dram_parameter("input2", [rows, DIM2], f32, isOutput=False)
    cgd = nc.declare_dram_parameter("cgmat", [DIM2, A_COLS], f32, isOutput=False)
    outd = nc.declare_dram_parameter("out", [rows, DIM_OUT], f32, isOutput=True)

    # chunk list: (row0, nb full blocks, p rows in last block)
    chunks = []
    r0 = 0
    while r0 < rows:
        nfull = min(B_BLOCKS, (rows - r0) // BLOCK_P)
        if nfull > 0:
            chunks.append((r0, nfull, BLOCK_P))
            r0 += nfull * BLOCK_P
        else:
            chunks.append((r0, 1, rows - r0))
            r0 = rows
    # benchmarking aid: repeat the full pass PASSES times (same outputs
    # rewritten) so on-device time can be isolated from host overhead
    chunks = chunks * int(os.environ.get("TP_KERNEL_PASSES", "1"))

    with TileContext(nc) as tc:
        with tc.tile_pool(name="const", bufs=1) as cpool, \
             tc.tile_pool(name="io", bufs=2) as iopool, \
             tc.tile_pool(name="tmp", bufs=2) as tmppool, \
             tc.tile_pool(name="psum", bufs=8, space="PSUM") as ppool:

            cgt = cpool.tile([DIM2, A_COLS], f32, tag="cg")
            nc.sync.dma_start(out=cgt[:], in_=cgd[:])

            for (row0, nb, pl) in chunks:
                # pl == BLOCK_P for all blocks except a lone partial chunk
                nrows = (nb - 1) * BLOCK_P + pl
                x1t = iopool.tile([BLOCK_P, nb * DIM1], f32, tag="x1")
                x2t = iopool.tile([DIM2, nb * BLOCK_P], f32, tag="x2t")
                x2r = iopool.tile([BLOCK_P, nb * DIM2], f32, tag="x2r")
                at = iopool.tile([BLOCK_P, nb * A_COLS], f32, tag="A")
                outt = iopool.tile([BLOCK_P, nb * DIM_OUT], f32, tag="out")

                src1 = x1d[row0:row0 + nrows, :].rearrange(
                    "(b n) f -> n b f", n=pl)
                src2 = x2d[row0:row0 + nrows, :].rearrange(
                    "(b n) j -> j b n", n=pl)
                src2r = x2d[row0:row0 + nrows, :].rearrange(
                    "(b n) j -> n b j", n=pl)
                dst_out = outd[row0:row0 + nrows, :].rearrange(
                    "(b n) f -> n b f", n=pl)

                x1v = x1t[:pl].rearrange("n (b f) -> n b f", b=nb)
                x2v = x2t[:].rearrange("j (b n) -> j b n", b=nb)[:, :, :pl]
                x2rv = x2r[:pl].rearrange("n (b j) -> n b j", b=nb)
                av = at[:pl].rearrange("n (b f) -> n b f", b=nb)
                outv = outt[:pl].rearrange("n (b f) -> n b f", b=nb)

                nc.sync.dma_start(out=x1v, in_=src1)
                nc.sync.dma_start(out=x2v, in_=src2)
                nc.sync.dma_start(out=x2rv, in_=src2r)

                # stage 1: A = x2 @ CGMAT per block (TensorE), evac by ScalarE
                for b in range(nb):
                    pt = ppool.tile([BLOCK_P, A_COLS], f32, tag="apsum")
                    nc.tensor.matmul(
                        out=pt[:pl, :],
                        lhsT=x2v[:, b, :],
                        rhs=cgt[:],
                        start=True, stop=True)
                    nc.scalar.copy(out=av[:, b, :], in_=pt[:pl, :])

                # stage 2: per-path ops (VectorE; l2=0 paths on ScalarE)
                for p, (s1, mul1, d1, s2, mul2, d2, l3, p3, cg) in enumerate(PATHS):
                    g = L1_OF_PATH[p]
                    d3 = 2 * l3 + 1
                    Kg = K_G[g]
                    plan = PLANS[p]
                    if plan[0][0] == 'act':
                        # out = x1_block * x2[:, 0]  (per-partition scale)
                        for b in range(nb):
                            nc.scalar.mul(
                                out=outv[:, b:b + 1, OO[p]:OO[p] + mul1 * d3],
                                in_=x1v[:, b:b + 1, s1:s1 + mul1 * d1],
                                mul=x2r[:pl, b * DIM2:b * DIM2 + 1])
                        continue
                    # out block view (n, b, u, k3)
                    ov = outv[:, :, OO[p]:OO[p] + mul1 * d3].rearrange(
                        "n b (u k) -> n b u k", k=d3)
                    # x1 block view (n, b, u, i)
                    xv = x1v[:, :, s1:s1 + mul1 * d1].rearrange(
                        "n b (u i) -> n b u i", i=d1)
                    # A rows for this group (n, b, i, k) ; path cols at KO[p]
                    gav = av[:, :, A_BASE[g]:A_BASE[g] + d1 * Kg].rearrange(
                        "n b (i k) -> n b i k", k=Kg)
                    if plan[0][0] == 'dense':
                        # all products into tmp (n,b,u,k,i), then reduce over i
                        tt = tmppool.tile(
                            [BLOCK_P, nb * mul1 * d3 * d1], f32, tag="scratch")
                        tv = tt[:pl].rearrange(
                            "n (b u k i) -> n b u k i", u=mul1, k=d3, i=d1)
                        xa = xv.unsqueeze(3).broadcast_to(
                            [pl, nb, mul1, d3, d1])
                        aa = gav[:, :, :, KO[p]:KO[p] + d3].transpose(
                            [0, 1, 3, 2]).unsqueeze(2).broadcast_to(
                            [pl, nb, mul1, d3, d1])
                        nc.vector.tensor_tensor(tv, xa, aa, mult)
                        nc.vector.tensor_reduce(
                            out=ov, in_=tv, axis=mybir.AxisListType.X, op=add)
                        continue
                    for (kind, i, a, ln, s) in plan:
                        o = ov[:, :, :, a:a + (ln - 1) * s + 1:s]
                        xa = xv[:, :, :, i:i + 1].broadcast_to(
                            [pl, nb, mul1, ln])
                        aa = gav[:, :, i,
                                 KO[p] + a:KO[p] + a + (ln - 1) * s + 1:s]
                        aa = aa.unsqueeze(2).broadcast_to([pl, nb, mul1, ln])
                        if kind == 'mw':
                            nc.vector.tensor_tensor(o, xa, aa, mult)
                        else:
                            tt = tmppool.tile(
                                [BLOCK_P, nb * mul1 * ln], f32, tag="scratch")
                            tv = tt[:pl].rearrange(
                                "n (b u k) -> n b u k", u=mul1, k=ln)
                            nc.vector.tensor_tensor(tv, xa, aa, mult)
                            nc.vector.tensor_tensor(o, o, tv, add)

                nc.sync.dma_start(out=dst_out, in_=outv)

    return nc


def _get_nc(rows):
    if rows not in _BUILD_CACHE:
        _BUILD_CACHE[rows] = _build_nc(rows)
    return _BUILD_CACHE[rows]


def kernel(input1, input2):
    from concourse.bass_utils import run_bass_kernel_spmd

    input1 = np.ascontiguousarray(np.asarray(input1, dtype=np.float32))
    input2 = np.ascontiguousarray(np.asarray(input2, dtype=np.float32))
    n = input1.shape[0]
    rows = -(-n // N_CORES)  # ceil
    pad = rows * N_CORES - n
    if pad:
        input1 = np.concatenate(
            [input1, np.zeros((pad, DIM1), np.float32)], axis=0)
        input2 = np.concatenate(
            [input2, np.zeros((pad, DIM2), np.float32)], axis=0)

    nc = _get_nc(rows)
    in_maps = [
        {
            "input1": input1[c * rows:(c + 1) * rows],
            "input2": input2[c * rows:(c + 1) * rows],
            "cgmat": CGMAT,
        }
        for c in range(N_CORES)
    ]
    res = run_bass_kernel_spmd(nc, in_maps, core_ids=list(range(N_CORES)))
    out = np.concatenate([res.results[c]["out"] for c in range(N_CORES)], axis=0)
    return out[:n]


# revision 17
# speedup vs baseline: 90.1585x; 45.3908x over previous
"""Trainium2 Bass kernel for nn_BaseFullTensorProduct (e3nn-style full tensor product).

out[n] = FullTensorProduct(input1[n], input2[n]) over 19 CG paths:
  IRREPS1 = 64x0e + 64x1o + 32x2e (dim 416), IRREPS2 = 1x0e + 1x1o + 1x2e (dim 9),
  output dim 3744, N = 50000 rows, pure data-parallel over 8 NeuronCores.

Per-core pipeline (n-on-partitions layout, features on the free dim):
  stage 1 (TensorE):  A[n, (i,k)] = sum_j CGMAT[j, (i,k)] * x2[n, j]
      one matmul per 128-row block: stationary = x2^T tile [9 x 128],
      moving = constant CGMAT [9 x 315], out lands [128 x 315] in PSUM
      (n on partitions), evacuated to SBUF by ScalarE.
  stage 2 (VectorE):  out[n, u*d3+k] = sum_i x1[n, u*d1+i] * A[n, (i,k)]
      broadcast-AP tensor_tensor mult/add ops, one statically planned op list
      per CG path exploiting the structural zeros of the CG tensors; final ops
      write directly into the e3nn-sorted output column layout.
  Output tiles [128 x B*3744] stored with one contiguous DMA per chunk.
"""
import os
import numpy as np
from fractions import Fraction
from math import factorial

# ----------------------------------------------------------------------------
# Problem constants (hardcoded per spec: do not read spec.json / reference.py)
# ----------------------------------------------------------------------------
IRREPS1 = [(64, 0, 1), (64, 1, -1), (32, 2, 1)]
IRREPS2 = [(1, 0, 1), (1, 1, -1), (1, 2, 1)]
DIM1 = 416
DIM2 = 9
DIM_OUT = 3744
N_CORES = 8
BLOCK_P = 128          # rows per partition-block
B_BLOCKS = 5           # blocks fused per chunk (amortizes DVE per-op overhead)


# ----------------------------------------------------------------------------
# Clebsch-Gordan construction (e3nn real-basis convention, exact rationals)
# ----------------------------------------------------------------------------
def _fact(n):
    return factorial(round(n))


def _su2_cg(j1, j2, j3, m1, m2, m3):
    if m3 != m1 + m2:
        return 0.0
    vmin = int(max(-j1 + j2 + m3, -j1 + m1, 0))
    vmax = int(min(j2 + j3 + m1, j3 - j1 + j2, j3 + m3))
    C = ((2 * j3 + 1) * Fraction(
        _fact(j3 + j1 - j2) * _fact(j3 - j1 + j2) * _fact(j1 + j2 - j3)
        * _fact(j3 + m3) * _fact(j3 - m3),
        _fact(j1 + j2 + j3 + 1) * _fact(j1 - m1) * _fact(j1 + m1)
        * _fact(j2 - m2) * _fact(j2 + m2))) ** 0.5
    S = 0
    for v in range(vmin, vmax + 1):
        S += (-1) ** (v + j2 + m2) * Fraction(
            _fact(j2 + j3 + m1 - v) * _fact(j1 - m1 + v),
            _fact(v) * _fact(j3 - j1 + j2 - v) * _fact(j3 + m3 - v)
            * _fact(v + j1 - j2 - m3))
    return float(C * S)


def _su2_cg_tensor(j1, j2, j3):
    mat = np.zeros((2 * j1 + 1, 2 * j2 + 1, 2 * j3 + 1))
    for i, m1 in enumerate(range(-j1, j1 + 1)):
        for j, m2 in enumerate(range(-j2, j2 + 1)):
            if abs(m1 + m2) <= j3:
                mat[i, j, j3 + m1 + m2] = _su2_cg(j1, j2, j3, m1, m2, m1 + m2)
    return mat


def _q(l):
    q = np.zeros((2 * l + 1, 2 * l + 1), dtype=np.complex128)
    for m in range(-l, 0):
        q[l + m, l + abs(m)] = 1 / np.sqrt(2)
        q[l + m, l - abs(m)] = -1j / np.sqrt(2)
    q[l, l] = 1.0
    for m in range(1, l + 1):
        q[l + m, l + abs(m)] = (-1) ** m / np.sqrt(2)
        q[l + m, l - abs(m)] = 1j * (-1) ** m / np.sqrt(2)
    return (-1j) ** l * q


def _clebsch_gordan(l1, l2, l3):
    C = _su2_cg_tensor(l1, l2, l3).astype(np.complex128)
    C = np.einsum('ij,kl,mn,ikn->jlm', _q(l1), _q(l2), np.conj(_q(l3).T), C)
    C = np.real(C)
    return C / np.linalg.norm(C)


def _build_paths():
    paths = []
    off1 = 0
    for mul1, l1, p1 in IRREPS1:
        d1 = 2 * l1 + 1
        off2 = 0
        for mul2, l2, p2 in IRREPS2:
            d2 = 2 * l2 + 1
            for l3 in range(abs(l1 - l2), l1 + l2 + 1):
                cg = _clebsch_gordan(l1, l2, l3) * np.sqrt(2 * l3 + 1)
                paths.append((off1, mul1, d1, off2, mul2, d2, l3, p1 * p2, cg))
            off2 += mul2 * d2
        off1 += mul1 * d1
    order = [i for i, _ in sorted(
        enumerate(paths), key=lambda t: (t[1][6], -t[1][7] * (-1) ** t[1][6]))]
    return paths, order


PATHS, ORDER = _build_paths()

# ----------------------------------------------------------------------------
# Derived metadata: groups, A-matrix layout, CGMAT, output offsets, op plans
# ----------------------------------------------------------------------------
# Groups by l1 block of input1: (mul1, d1, x1 col base)
GROUPS = [(64, 1, 0), (64, 3, 64), (32, 5, 256)]
L1_OF_PATH = [{1: 0, 3: 1, 5: 2}[p[2]] for p in PATHS]

K_G = [0, 0, 0]
for p, pa in enumerate(PATHS):
    K_G[L1_OF_PATH[p]] += 2 * pa[6] + 1
assert K_G == [9, 27, 45]
A_BASE = [0, K_G[0] * 1, K_G[0] * 1 + K_G[1] * 3]
A_COLS = A_BASE[2] + K_G[2] * 5
assert A_COLS == 315

KO = []
_run = [0, 0, 0]
for p, pa in enumerate(PATHS):
    g = L1_OF_PATH[p]
    KO.append(_run[g])
    _run[g] += 2 * pa[6] + 1

OO = [0] * len(PATHS)
_off = 0
for p in ORDER:
    pa = PATHS[p]
    OO[p] = _off
    _off += pa[1] * (2 * pa[6] + 1)
assert _off == DIM_OUT


def _build_cgmat():
    m = np.zeros((DIM2, A_COLS), dtype=np.float32)
    for p, (s1, mul1, d1, s2, mul2, d2, l3, p3, cg) in enumerate(PATHS):
        g = L1_OF_PATH[p]
        d3 = 2 * l3 + 1
        for i in range(d1):
            for j2 in range(d2):
                for k3 in range(d3):
                    m[s2 + j2, A_BASE[g] + i * K_G[g] + KO[p] + k3] = cg[i, j2, k3]
    return m


CGMAT = _build_cgmat()


def _arith_runs(ks):
    """Cover a sorted k-set with maximal arithmetic runs, stride in {1, 2}."""
    ks = list(ks)
    runs = []
    while ks:
        a = ks[0]
        l1 = 1
        while a + l1 in ks:
            l1 += 1
        l2 = 1
        while a + 2 * l2 in ks:
            l2 += 1
        if l2 > l1:
            runs.append((a, l2, 2))
            for t in range(l2):
                ks.remove(a + 2 * t)
        else:
            runs.append((a, l1, 1))
            for t in range(l1):
                ks.remove(a + t)
    return runs


_DVE_INIT = 58  # per-op SBUF access bubble (cycles) used for plan costing


def _plan_path(p):
    """Static op plan for path p from the CG (i,k) sparsity.
    ops: [('act',)]                        -> l2=0: ScalarE out = x1_block * x2[:,0]
         [('dense',)]                      -> dense mult + tensor_reduce over i
         ('mw', i, a, ln, s)               -> TT mult-write, k = a, a+s, ...
         ('mta', i, a, ln, s)              -> TT mult-to-tmp + TT add
    """
    (s1, mul1, d1, s2, mul2, d2, l3, p3, cg) = PATHS[p]
    d3 = 2 * l3 + 1
    nz = (np.abs(cg) > 1e-12).any(axis=1)
    if d2 == 1:
        # cg for coupling with a scalar irrep is exactly the identity map
        assert np.allclose(cg[:, 0, :], np.eye(d1)), f"path {p} l2=0 cg != I"
        return [('act',)]
    if d1 == 1:
        # scalar x1 block: cg[0] must be the identity over (j, k)
        assert d2 == d3 and np.allclose(cg[0], np.eye(d2)), f"path {p} cg != I"
        return [('actk',)]
    ops = []
    written = np.zeros(d3, dtype=bool)
    order = sorted(range(d1), key=lambda i: -int(nz[i].sum()))
    for i in order:
        ks = [int(k) for k in np.flatnonzero(nz[i])]
        if not ks:
            continue
        for (a, ln, s) in _arith_runs(ks):
            j = 0
            while j < ln:
                w = bool(written[a + j * s])
                e = j
                while e < ln and bool(written[a + e * s]) == w:
                    e += 1
                ops.append(('mta' if w else 'mw', i, a + j * s, e - j, s))
                j = e
        for k in ks:
            written[k] = True
    assert written.all()
    # dense alternative: one mult over (u, k, i) + one tensor_reduce over i
    B = B_BLOCKS
    run_cost = sum(
        (2 if o[0] == 'mta' else 1) * (B * mul1 * o[3] + _DVE_INIT) for o in ops)
    dense_cost = 2 * (B * mul1 * d1 * d3) + 2 * _DVE_INIT
    if dense_cost < run_cost:
        return [('dense',)]
    return ops


PLANS = [_plan_path(p) for p in range(len(PATHS))]

# ----------------------------------------------------------------------------
# Bass kernel
# ----------------------------------------------------------------------------
_BUILD_CACHE = {}


def _patch_tile_drain():
    """The walrus in this env allows only ONE sem-wait per instruction
    ("Too many sync wait commands"); Tile freely emits multi-wait
    instructions. Split extra waits onto same-engine nops that execute
    just before the instruction (same semantics: engine streams are
    in-order, so waiting earlier on the same engine is equivalent)."""
    import bass_rust
    import concourse.mybir as mybir
    import concourse.tile as tile
    from concourse.vector_clock import ScopedClock

    if getattr(tile.TileContext, "_drain_split_patched", False):
        return

    _orig_add = tile.TileContext._add_instruction

    def _add_instruction(self, inst):
        si = inst.sync_info
        if si is not None and si.on_wait and len(si.on_wait) > 1:
            waits = list(si.on_wait)
            si.on_wait = waits[-1:]
            for w in waits[:-1]:
                nop = mybir.InstNoOp(
                    name=self.nc.get_next_instruction_name(), ins=[], outs=[])
                nop.engine = inst.engine
                nop.sync_info = bass_rust.SyncInfo(on_wait=[w], on_update=[])
                _orig_add(self, nop)
        _orig_add(self, inst)

    def _drain_and_barrier(self, tick_clock, wait_clock):
        holder = self.nc.sync.nop()
        wait_clock.add_sem_waits(
            holder.ins, ScopedClock({None: tick_clock.global_clock})
        )
        si = holder.ins.sync_info
        waits = list(si.on_wait or []) if si is not None else []
        if len(waits) > 1:
            si.on_wait = waits[:1]
            for w in waits[1:]:
                nop = self.nc.sync.nop()
                nop.ins.sync_info = bass_rust.SyncInfo(on_wait=[w], on_update=[])
        self.nc.sync.drain()
        self.nc.all_engine_barrier()
        assert self.sems is not None
        popped = self.nc._tile_sem_poison_stack.pop()
        assert popped is self._sem_poison
        self.nc.clear_and_free_semaphores(list(self.sems.allocated().values()))
        self.nc.all_engine_barrier()

    tile.TileContext._add_instruction = _add_instruction
    tile.TileContext._drain_and_barrier = _drain_and_barrier
    tile.TileContext._drain_split_patched = True


def _build_nc(rows):
    """Build the per-core Bass kernel for `rows` rows."""
    import concourse.bass as bass
    import concourse.mybir as mybir
    from concourse.tile import TileContext

    _patch_tile_drain()
    f32 = mybir.dt.float32
    mult = mybir.AluOpType.mult
    add = mybir.AluOpType.add

    nc = bass.Bass(num_devices=N_CORES)
    x1d = nc.declare_dram_parameter("input1", [rows, DIM1], f32, isOutput=False)
    x2d = nc.declare_dram_parameter("input2", [rows, DIM2], f32, isOutput=False)
    cgd = nc.declare_dram_parameter("cgmat", [DIM2, A_COLS], f32, isOutput=False)
    outd = nc.declare_dram_parameter("out", [rows, DIM_OUT], f32, isOutput=True)

    # chunk list: (row0, nb blocks, valid rows).  The tail chunk computes on
    # full 128-row blocks (junk in the unused partitions of the last block —
    # finite stale SBUF data, never stored) and DMAs only the valid rows.
    chunks = []
    r0 = 0
    while r0 < rows:
        rem = rows - r0
        nfull = min(B_BLOCKS, rem // BLOCK_P)
        if nfull > 0:
            chunks.append((r0, nfull, nfull * BLOCK_P))
            r0 += nfull * BLOCK_P
        else:
            chunks.append((r0, 1, rem))
            r0 = rows
    # benchmarking aid: repeat the full pass PASSES times (same outputs
    # rewritten) so on-device time can be isolated from host overhead
    chunks = chunks * int(os.environ.get("TP_KERNEL_PASSES", "1"))

    with TileContext(nc) as tc:
        with tc.tile_pool(name="const", bufs=1) as cpool, \
             tc.tile_pool(name="io", bufs=2) as iopool, \
             tc.tile_pool(name="tmp", bufs=3) as tmppool, \
             tc.tile_pool(name="psum", bufs=8, space="PSUM") as ppool:

            cgt = cpool.tile([DIM2, A_COLS], f32, tag="cg")
            nc.sync.dma_start(out=cgt[:], in_=cgd[:])

            for (row0, nb, nvalid) in chunks:
                pl = BLOCK_P  # compute always runs on full partitions
                nfull = nvalid // BLOCK_P
                prem = nvalid % BLOCK_P
                x1t = iopool.tile([BLOCK_P, nb * DIM1], f32, tag="x1")
                x2t = iopool.tile([DIM2, nb * BLOCK_P], f32, tag="x2t")
                x2r = iopool.tile([BLOCK_P, nb * DIM2], f32, tag="x2r")
                at = iopool.tile([BLOCK_P, nb * A_COLS], f32, tag="A")
                outt = iopool.tile([BLOCK_P, nb * DIM_OUT], f32, tag="out")

                x1v = x1t[:].rearrange("n (b f) -> n b f", b=nb)
                x2v = x2t[:].rearrange("j (b n) -> j b n", b=nb)
                x2rv = x2r[:].rearrange("n (b j) -> n b j", b=nb)
                av = at[:].rearrange("n (b f) -> n b f", b=nb)
                outv = outt[:].rearrange("n (b f) -> n b f", b=nb)

                # DMA the full blocks and (tail chunk only) the partial block
                def _io(dram, view, store=False):
                    parts = []
                    if nfull:
                        parts.append((
                            dram[row0:row0 + nfull * BLOCK_P, :],
                            BLOCK_P, view[:, 0:nfull, :]))
                    if prem:
                        r = row0 + nfull * BLOCK_P
                        parts.append((
                            dram[r:r + prem, :], prem,
                            view[:prem, nfull:nfull + 1, :]))
                    return parts

                for (dsrc, n_, tdst) in _io(x1d, x1v):
                    nc.sync.dma_start(
                        out=tdst, in_=dsrc.rearrange("(b n) f -> n b f", n=n_))
                for (dsrc, n_, tdst) in _io(x2d, x2rv):
                    nc.sync.dma_start(
                        out=tdst, in_=dsrc.rearrange("(b n) j -> n b j", n=n_))
                if nfull:
                    nc.sync.dma_start(
                        out=x2v[:, 0:nfull, :],
                        in_=x2d[row0:row0 + nfull * BLOCK_P, :].rearrange(
                            "(b n) j -> j b n", n=BLOCK_P))
                if prem:
                    r = row0 + nfull * BLOCK_P
                    nc.sync.dma_start(
                        out=x2v[:, nfull:nfull + 1, :prem],
                        in_=x2d[r:r + prem, :].rearrange(
                            "(b n) j -> j b n", n=prem))

                # stage 1: A = x2 @ CGMAT per block (TensorE), evac by ScalarE
                for b in range(nb):
                    pt = ppool.tile([BLOCK_P, A_COLS], f32, tag="apsum")
                    nc.tensor.matmul(
                        out=pt[:, :],
                        lhsT=x2v[:, b, :],
                        rhs=cgt[:],
                        start=True, stop=True)
                    nc.scalar.copy(out=av[:, b, :], in_=pt[:, :])

                # stage 2: per-path ops (VectorE; l2=0 paths on ScalarE)
                for p, (s1, mul1, d1, s2, mul2, d2, l3, p3, cg) in enumerate(PATHS):
                    g = L1_OF_PATH[p]
                    d3 = 2 * l3 + 1
                    Kg = K_G[g]
                    plan = PLANS[p]
                    if plan[0][0] == 'act':
                        # out = x1_block * x2[:, 0]  (per-partition scale)
                        for b in range(nb):
                            nc.scalar.mul(
                                out=outv[:, b:b + 1, OO[p]:OO[p] + mul1 * d3],
                                in_=x1v[:, b:b + 1, s1:s1 + mul1 * d1],
                                mul=x2r[:, b * DIM2:b * DIM2 + 1])
                        continue
                    if plan[0][0] == 'actk':
                        # d1 == 1, cg = I: out[:, u*d3+k] = x1[:, u]*x2[:, s2+k]
                        for b in range(nb):
                            for k in range(d3):
                                nc.scalar.mul(
                                    out=outv[:, b:b + 1,
                                             OO[p] + k:OO[p] + mul1 * d3:d3],
                                    in_=x1v[:, b:b + 1, s1:s1 + mul1],
                                    mul=x2r[:, b * DIM2 + s2 + k
                                            : b * DIM2 + s2 + k + 1])
                        continue
                    # out block view (n, b, u, k3)
                    ov = outv[:, :, OO[p]:OO[p] + mul1 * d3].rearrange(
                        "n b (u k) -> n b u k", k=d3)
                    # x1 block view (n, b, u, i)
                    xv = x1v[:, :, s1:s1 + mul1 * d1].rearrange(
                        "n b (u i) -> n b u i", i=d1)
                    # A rows for this group (n, b, i, k) ; path cols at KO[p]
                    gav = av[:, :, A_BASE[g]:A_BASE[g] + d1 * Kg].rearrange(
                        "n b (i k) -> n b i k", k=Kg)
                    if plan[0][0] == 'dense':
                        # all products into tmp (n,b,u,k,i), then reduce over i
                        tt = tmppool.tile(
                            [BLOCK_P, nb * mul1 * d3 * d1], f32, tag="scratch")
                        tv = tt[:pl].rearrange(
                            "n (b u k i) -> n b u k i", u=mul1, k=d3, i=d1)
                        xa = xv.unsqueeze(3).broadcast_to(
                            [pl, nb, mul1, d3, d1])
                        aa = gav[:, :, :, KO[p]:KO[p] + d3].transpose(
                            [0, 1, 3, 2]).unsqueeze(2).broadcast_to(
                            [pl, nb, mul1, d3, d1])
                        nc.vector.tensor_tensor(tv, xa, aa, mult)
                        nc.vector.tensor_reduce(
                            out=ov, in_=tv, axis=mybir.AxisListType.X, op=add)
                        continue
                    for (kind, i, a, ln, s) in plan:
                        o = ov[:, :, :, a:a + (ln - 1) * s + 1:s]
                        xa = xv[:, :, :, i:i + 1].broadcast_to(
                            [pl, nb, mul1, ln])
                        aa = gav[:, :, i,
                                 KO[p] + a:KO[p] + a + (ln - 1) * s + 1:s]
                        aa = aa.unsqueeze(2).broadcast_to([pl, nb, mul1, ln])
                        if kind == 'mw':
                            nc.vector.tensor_tensor(o, xa, aa, mult)
                        else:
                            tt = tmppool.tile(
                                [BLOCK_P, nb * mul1 * ln], f32, tag="scratch")
                            tv = tt[:pl].rearrange(
                                "n (b u k) -> n b u k", u=mul1, k=ln)
                            nc.vector.tensor_tensor(tv, xa, aa, mult)
                            nc.vector.tensor_tensor(o, o, tv, add)

                for (dsrc, n_, tsrc) in _io(outd, outv):
                    nc.sync.dma_start(
                        out=dsrc.rearrange("(b n) f -> n b f", n=n_), in_=tsrc)

    return nc


def _get_nc(rows):
    if rows not in _BUILD_CACHE:
        _BUILD_CACHE[rows] = _build_nc(rows)
    return _BUILD_CACHE[rows]


def kernel(input1, input2):
    from concourse.bass_utils import run_bass_kernel_spmd

    input1 = np.ascontiguousarray(np.asarray(input1, dtype=np.float32))
    input2 = np.ascontiguousarray(np.asarray(input2, dtype=np.float32))
    n = input1.shape[0]
    rows = -(-n // N_CORES)  # ceil
    pad = rows * N_CORES - n
    if pad:
        input1 = np.concatenate(
            [input1, np.zeros((pad, DIM1), np.float32)], axis=0)
        input2 = np.concatenate(
            [input2, np.zeros((pad, DIM2), np.float32)], axis=0)

    nc = _get_nc(rows)
    in_maps = [
        {
            "input1": input1[c * rows:(c + 1) * rows],
            "input2": input2[c * rows:(c + 1) * rows],
            "cgmat": CGMAT,
        }
        for c in range(N_CORES)
    ]
    res = run_bass_kernel_spmd(nc, in_maps, core_ids=list(range(N_CORES)))
    out = np.concatenate([res.results[c]["out"] for c in range(N_CORES)], axis=0)
    return out[:n]
